# revision 33
# baseline (speedup 1.0000x reference)
"""Trainium2 Bass kernel for a dense transformer block (pre-LN, causal MHA + FFN).

Reference computation (B=256, T=256, C=384, H=6, hd=64, D_FF=1536):
    h  = LN(x; g1, b1) ; q,k,v = per-head h @ W{q,k,v}
    wei = softmax(causal(q @ k^T * sqrt(C)))
    sa  = concat_heads(wei @ v) @ w_proj + b_proj ; x = x + sa
    h2  = LN(x; g2, b2) ; out = x + relu(h2 @ w1 + b1) @ w2 + b2

Sharding: pure data-parallel over batch B across 8 NeuronCores (32 seqs/core).
Weights replicated; LN gains/biases and the sqrt(C) scale folded host-side.

Design notes:
  - Emission is software-pipelined across sequences: seq s+1's x prefetch,
    LN1 and h^T weave into seq s's attention pairs; s+1's QKV matmuls weave
    into s's proj/FFN stages. The PE instruction stream never drains, which
    also keeps the HAM clock gate at 2.4 GHz (an idle-ish PE gets throttled
    to 1.2 GHz within ~3.4us).
  - Softmax row-sums come from the PE: v carries a ones column per head, so
    each att matmul accumulates the row-sum into PSUM col 64 — no ACT
    accumulator reads, exp has no accum_out.
  - Causal mask: -1e9 additive mask on the two diagonal blocks only (the
    off-diagonal block of token-tile 1 is fully valid); masks and row-maxes
    run 2-heads-batched on DVE in PSUM.
  - PSUM tags are split (S-tiles / transpose staging / ffn+proj conveyor /
    att accumulators) so the three attention pairs only serialize on their
    own exps instead of each other's whole softmax chains — this single
    change was worth ~15% and keeps the HAM clock 93% warm.
  - FFN runs in fp8-e4m3 DoubleRow (weights pre-scaled by 64 to clear the
    e4m3 subnormal floor; the 2^-12 factor is undone in the final residual
    via scalar_tensor_tensor). FFN1's K=384 is zero-padded to 512 = two
    DoubleRow k-tile pairs; FFN2's K=1536 is six exact pairs.
  - LN rstd = exp(-0.5*ln(var+eps)) on ACT. Ln/Exp/Copy/Relu share one
    activation-table set — zero table thrash (Sqrt would thrash vs Exp).
  - Transposes (h, wei, att, h2) on the PE via identity matmuls (DMA-xbar
    transpose measured ~1.5-2.4us fixed cost per instruction here — too
    slow). PSUM->SBUF copies split between ACT and DVE to balance load.
  - Compute dtype bf16 on the PE, fp32 PSUM accumulation, fp32 residual
    spine and softmax logits.
"""

import sys

for _p in ("/opt/trn_rl_repo", "/opt/pypackages"):
    if _p not in sys.path:
        sys.path.append(_p)

import numpy as np
import ml_dtypes

import concourse.bass as bass
import concourse.mybir as mybir
import concourse.tile as tile
from concourse.bass_utils import run_bass_kernel_spmd

# Problem constants (hardcoded per harness contract).
B, T, C = 256, 256, 384
H, HD = 6, 64
DFF = 4 * C  # 1536
SCALE = float(C) ** 0.5
LN_EPS = 1e-5
N_CORES = 8
B_SH = B // N_CORES          # 32 seqs per core
TOK = B_SH * T               # 8192 tokens per core
P = 128                      # partitions
CCH = C // P                 # 3 contraction chunks of 128
NPAIR = C // P               # 3 head-pairs (2 heads of 64 = 128 cols)
NFF = DFF // P               # 12 ff groups
VW = 66                      # per-head v stride: 64 vals + 1 ones col + pad

F32 = mybir.dt.float32
BF16 = mybir.dt.bfloat16
FP8 = mybir.dt.float8e4
FF_SCALE = 64.0  # fp8 weight pre-scale; folded out as 2^-12 in the residual

_BF = ml_dtypes.bfloat16
_F8 = ml_dtypes.float8_e4m3fn

_CACHE = {}


def _hoist_extra_waits(nc):
    """This container's walrus supports one sync-wait per instruction; Tile
    attaches several. Hoist all-but-one onto NoOps on the same engine just
    before the instruction (engine-order preserving, deadlock-free since
    every sem's producer precedes the consumer in Tile's global schedule)."""
    for f in nc.m.functions:
        for blk in f.blocks:
            new_insts, dirty = [], False
            for ins in blk.instructions:
                si = ins.sync_info
                waits = list(si.on_wait) if (si is not None and si.on_wait) else []
                if len(waits) > 1:
                    for w in waits[:-1]:
                        nop = mybir.InstNoOp(name=f"wsplit_{nc.next_id()}")
                        nop.engine = ins.engine
                        nop.sync_info = mybir.SyncInfo(on_wait=[w], on_update=[])
                        nc.inst_map[nop.name] = nop
                        new_insts.append(nop)
                    ins.sync_info = mybir.SyncInfo(
                        on_wait=[waits[-1]],
                        on_update=list(si.on_update) if si.on_update else [],
                    )
                    dirty = True
                new_insts.append(ins)
            if dirty:
                blk.instructions = new_insts


def _build(has_bv, has_bp, has_b2, has_b1, has_bqk):
    nc = bass.Bass()

    x_h = nc.declare_dram_parameter("x", [TOK, C], F32, isOutput=False)
    wq_h = nc.declare_dram_parameter("wq_m", [C, C], BF16, isOutput=False)
    wk_h = nc.declare_dram_parameter("wk_m", [C, C], BF16, isOutput=False)
    wv_h = nc.declare_dram_parameter("wv_m", [C, C], BF16, isOutput=False)
    wp_h = nc.declare_dram_parameter("wp_m", [C, C], BF16, isOutput=False)
    w1_h = nc.declare_dram_parameter("w1_m", [4 * P, DFF], FP8, isOutput=False)
    w2_h = nc.declare_dram_parameter("w2_m", [DFF, C], FP8, isOutput=False)
    bq_h = nc.declare_dram_parameter("bq_v", [C], F32, isOutput=False)
    bk_h = nc.declare_dram_parameter("bk_v", [C], F32, isOutput=False)
    bext_h = nc.declare_dram_parameter("bext_v", [3, C], BF16, isOutput=False)
    b1_h = nc.declare_dram_parameter("b1_v", [DFF], F32, isOutput=False)
    iden_h = nc.declare_dram_parameter("iden_m", [P, P], BF16, isOutput=False)
    mask_h = nc.declare_dram_parameter("mask_m", [P, P], F32, isOutput=False)
    tri_h = nc.declare_dram_parameter("tri_m", [P, P], BF16, isOutput=False)
    out_h = nc.declare_dram_parameter("out", [TOK, C], F32, isOutput=True)

    AX = mybir.AxisListType
    OP = mybir.AluOpType
    AF = mybir.ActivationFunctionType

    with tile.TileContext(nc) as tc:
        with (
            tc.tile_pool(name="const", bufs=1) as cst,
            tc.tile_pool(name="xs", bufs=8) as xp,
            tc.tile_pool(name="acts", bufs=6) as ap,
            tc.tile_pool(name="qkv", bufs=3) as qkvp,
            tc.tile_pool(name="attn", bufs=4) as atp,
            tc.tile_pool(name="stats", bufs=24) as stp,
            tc.tile_pool(name="ffn", bufs=3) as ffp,
            tc.tile_pool(name="outs", bufs=6) as op_,
            tc.tile_pool(name="ps_sa", bufs=2, space="PSUM") as ps_sa,
            tc.tile_pool(name="ps_st", bufs=2, space="PSUM") as ps_st,
            tc.tile_pool(name="ps_att", bufs=2, space="PSUM") as ps_ap,
            tc.tile_pool(name="ps_w", bufs=2, space="PSUM") as ps_w,
        ):
            # ---- constants / weights (resident) ----
            wq_sb = cst.tile([P, CCH, C], BF16)
            nc.gpsimd.dma_start(out=wq_sb, in_=wq_h[:].rearrange("(o p) f -> p o f", p=P))
            wk_sb = cst.tile([P, CCH, C], BF16)
            nc.gpsimd.dma_start(out=wk_sb, in_=wk_h[:].rearrange("(o p) f -> p o f", p=P))
            wv_sb = cst.tile([P, CCH, C], BF16)
            nc.gpsimd.dma_start(out=wv_sb, in_=wv_h[:].rearrange("(o p) f -> p o f", p=P))
            wp_sb = cst.tile([P, CCH, C], BF16)
            nc.gpsimd.dma_start(out=wp_sb, in_=wp_h[:].rearrange("(o p) f -> p o f", p=P))
            w1_sb = cst.tile([P, 4, DFF], FP8)
            nc.gpsimd.dma_start(out=w1_sb, in_=w1_h[:].rearrange("(o p) f -> p o f", p=P))
            w2_sb = cst.tile([P, NFF, C], FP8)
            nc.gpsimd.dma_start(out=w2_sb, in_=w2_h[:].rearrange("(o p) f -> p o f", p=P))
            bq_sb = cst.tile([HD, H], F32)
            nc.gpsimd.dma_start(out=bq_sb, in_=bq_h[:].rearrange("(o p) -> p o", p=HD))
            bk_sb = cst.tile([HD, H], F32)
            nc.gpsimd.dma_start(out=bk_sb, in_=bk_h[:].rearrange("(o p) -> p o", p=HD))
            b1_sb = cst.tile([P, NFF], F32)
            nc.gpsimd.dma_start(out=b1_sb, in_=b1_h[:].rearrange("(o p) -> p o", p=P))
            mask_sb = cst.tile([P, P], F32)
            nc.gpsimd.dma_start(out=mask_sb, in_=mask_h[:])
            tri_sb = cst.tile([P, P], BF16)
            nc.gpsimd.dma_start(out=tri_sb, in_=tri_h[:])
            iden_sb = cst.tile([P, P], BF16)
            nc.gpsimd.dma_start(out=iden_sb, in_=iden_h[:])
            eps_sb = cst.tile([P, 1], F32)
            nc.vector.memset(eps_sb, LN_EPS)
            # ones row + free-dim biases for the broadcast-bias matmul trick
            ones_sb = cst.tile([1, P], BF16)
            nc.vector.memset(ones_sb, 1.0)
            bext_sb = cst.tile([1, 3, C], BF16)
            nc.gpsimd.dma_start(
                out=bext_sb, in_=bext_h[:].rearrange("o (u f) -> u o f", u=1)
            )

            def bcast(ap_obj, n):
                """Append a stride-0 free dim of size n (free-dim broadcast)."""
                return bass.AP(
                    tensor=ap_obj.tensor, offset=ap_obj.offset,
                    ap=[*ap_obj.ap, [0, n]],
                )

            def bcast2(ap_obj, n):
                """Insert a stride-0 free dim of size n after the partition
                dim: [P, F] -> [P, n, F]."""
                return bass.AP(
                    tensor=ap_obj.tensor, offset=ap_obj.offset,
                    ap=[ap_obj.ap[0], [0, n], *ap_obj.ap[1:]],
                )

            def layer_norm(x_in, h_out):
                """h_out (bf16) = (x_in - mean) * rsqrt(var + eps); gains and
                biases are folded into downstream weights. rstd on ACT via
                exp(-0.5*ln(var+eps)) — same table set as the softmax Exp.
                The whole chain gates every downstream matmul of its seq, so
                schedule it ahead of bulk engine work."""
                with tc.high_priority(offset=600):
                    mv = stp.tile([P, 6], F32, tag="bnstats")
                    nc.vector.bn_stats(out=mv, in_=x_in)
                    agg = stp.tile([P, 2], F32, tag="bnagg")
                    nc.vector.bn_aggr(out=agg, in_=mv)
                    lnv = stp.tile([P, 1], F32, tag="lnv")
                    nc.scalar.activation(
                        out=lnv, in_=agg[:, 1:2], func=AF.Ln, bias=eps_sb,
                        scale=1.0
                    )
                    rstd = stp.tile([P, 1], F32, tag="rstd")
                    nc.scalar.activation(
                        out=rstd, in_=lnv, func=AF.Exp, bias=0.0, scale=-0.5
                    )
                    nc.vector.tensor_scalar(
                        out=h_out, in0=x_in,
                        scalar1=agg[:, 0:1], scalar2=rstd,
                        op0=OP.subtract, op1=OP.mult,
                    )

            def transpose3(src_bf16, dst_sb, nch, name, on_act):
                """dst_sb[:, c, :] (bf16 [P, nch, P]) = 128x128 transposes of
                src via PE; one batched PSUM->SBUF copy on ACT or DVE."""
                ps = ps_st.tile([P, 1024], BF16, tag="St", name=f"tp_{name}")
                psv = ps[:, 0:nch * P].rearrange("p (c t) -> p c t", c=nch)
                for c in range(nch):
                    nc.tensor.transpose(
                        psv[:, c, :], src_bf16[:, c * P:(c + 1) * P], iden_sb
                    )
                if on_act:
                    nc.scalar.activation(out=dst_sb, in_=psv, func=AF.Copy,
                                         bias=0.0, scale=1.0)
                else:
                    nc.vector.tensor_copy(out=dst_sb, in_=psv)

            # ---------------- per-seq stage emitters ----------------
            st = [dict() for _ in range(B_SH)]  # per-seq handles

            def x_prefetch(s):
                d = st[s]
                d["x"] = []
                for j in range(2):
                    it = 2 * s + j
                    x_i = xp.tile([P, C], F32, tag="x", name=f"x_{s}_{j}")
                    nc.gpsimd.dma_start(out=x_i, in_=x_h[it * P:(it + 1) * P, :])
                    d["x"].append(x_i)

            def a1_tile(s, j):
                """LN1 + h^T for token tile j of seq s."""
                d = st[s]
                if j == 0:
                    d["hT"] = ap.tile([P, CCH, T], BF16, tag="hT", bufs=3,
                                      name=f"hT_{s}")
                h_i = ap.tile([P, C], BF16, tag="h", name=f"h_{s}_{j}")
                layer_norm(d["x"][j], h_i)
                transpose3(h_i, d["hT"][:, :, j * P:(j + 1) * P], CCH,
                           f"h_{s}_{j}", on_act=True)

            def a2_v(s):
                """v for both tiles; v_ext carries a ones col per head for the
                PE-side softmax row-sums."""
                d = st[s]
                v_ext = qkvp.tile([P, 2, H, VW], BF16, tag="v", name=f"v_{s}")
                d["v"] = v_ext
                for j in range(2):
                    ps_v = ps_w.tile([P, 512], F32, tag="w", name=f"psv_{s}_{j}")
                    for c in range(CCH):
                        nc.tensor.matmul(
                            ps_v[:, 0:C], d["hT"][:, c, j * P:(j + 1) * P],
                            wv_sb[:, c, :],
                            start=(c == 0), stop=(c == CCH - 1 and not has_bv),
                        )
                    if has_bv:
                        nc.tensor.matmul(ps_v[:, 0:C], ones_sb, bext_sb[:, 0, :],
                                         start=False, stop=True)
                    nc.scalar.activation(
                        out=v_ext[:, j, :, 0:HD],
                        in_=ps_v[:, 0:C].rearrange("p (h c) -> p h c", h=H),
                        func=AF.Copy, bias=0.0, scale=1.0,
                    )
                    nc.vector.memset(v_ext[:, j, :, HD:HD + 1], 1.0)

            def a2_qk(s, which):
                """q^T or k^T: weight head-cols stationary (M=64), h^T moving
                at N=256; PSUM->SBUF cast on ACT."""
                d = st[s]
                w_sb, b_sb = (wq_sb, bq_sb) if which == "q" else (wk_sb, bk_sb)
                dst = qkvp.tile([HD, H, T], BF16, tag=which + "T",
                                name=f"{which}T_{s}")
                d[which + "T"] = dst
                for pr in range(NPAIR):
                    ps_qp = ps_w.tile([HD, 512], F32, tag="w",
                                      name=f"ps{which}{pr}_{s}")
                    for i in range(2):
                        hh = 2 * pr + i
                        for c in range(CCH):
                            nc.tensor.matmul(
                                ps_qp[:, i * T:(i + 1) * T],
                                w_sb[:, c, hh * HD:(hh + 1) * HD],
                                d["hT"][:, c, :],
                                start=(c == 0), stop=(c == CCH - 1),
                            )
                    if has_bqk:
                        for i in range(2):
                            hh = 2 * pr + i
                            nc.scalar.activation(
                                out=dst[:, hh, :],
                                in_=ps_qp[:, i * T:(i + 1) * T],
                                func=AF.Copy, bias=b_sb[:, hh:hh + 1], scale=1.0,
                            )
                    else:
                        nc.scalar.activation(
                            out=dst[:, 2 * pr:2 * pr + 2, :],
                            in_=ps_qp[:, 0:2 * T].rearrange("p (i t) -> p i t", i=2),
                            func=AF.Copy, bias=0.0, scale=1.0,
                        )

            def b_pair(s, pr):
                """Attention for head-pair pr: S matmuls, mask (tile0 only),
                row-maxes, exp, zero tile1's invalid diag entries, wei^T via
                PE, att matmuls (+ rowsum col)."""
                d = st[s]
                if pr == 0:
                    d["ps_att"] = [
                        ps_ap.tile([P, 512], F32, tag="attacc",
                                   name=f"ps_att{j}_{s}")
                        for j in range(2)
                    ]
                qT, kT = d["qT"], d["kT"]
                ps0 = ps_sa.tile([P, 512], F32, tag="Sa", name=f"s0_{s}_{pr}")
                ps1 = ps_sa.tile([P, 512], F32, tag="Sa", name=f"s1_{s}_{pr}")
                for i in range(2):
                    hh = 2 * pr + i
                    nc.tensor.matmul(ps0[:, i * P:(i + 1) * P],
                                     qT[:, hh, 0:P], kT[:, hh, 0:P],
                                     start=True, stop=True)
                    nc.tensor.matmul(ps1[:, i * T:(i + 1) * T],
                                     qT[:, hh, P:T], kT[:, hh, 0:T],
                                     start=True, stop=True)
                nmax0 = stp.tile([P, 2], F32, tag="nmax0")
                nmax1 = stp.tile([P, 2], F32, tag="nmax1")
                ps0v = ps0[:, 0:2 * P].rearrange("p (i c) -> p i c", i=2)
                ps1v = ps1[:, 0:2 * T].rearrange("p (i c) -> p i c", i=2)
                nc.vector.tensor_tensor(out=ps0v, in0=ps0v,
                                        in1=bcast2(mask_sb, 2), op=OP.add)
                nc.vector.tensor_reduce(out=nmax0, in_=ps0v, axis=AX.X,
                                        op=OP.max, negate=True)
                # tile0's exp can start here; tile1's mask/max follows
                nc.vector.tensor_tensor(out=ps1v[:, :, P:T],
                                        in0=ps1v[:, :, P:T],
                                        in1=bcast2(mask_sb, 2), op=OP.add)
                nc.vector.tensor_reduce(out=nmax1, in_=ps1v, axis=AX.X,
                                        op=OP.max, negate=True)
                # wei layout: [P, i, slot, P]; slot 0 = tile0 diag (s 0:128),
                # slots 1,2 = tile1 (s 0:128, 128:256)
                wei = atp.tile([P, 2, CCH, P], BF16, tag="wei",
                               name=f"wei_{s}_{pr}")
                for i in range(2):
                    nc.scalar.activation(
                        out=wei[:, i, 0, :], in_=ps0[:, i * P:(i + 1) * P],
                        func=AF.Exp, bias=nmax0[:, i:i + 1], scale=1.0,
                    )
                    nc.scalar.activation(
                        out=wei[:, i, 1:3, :].rearrange("p a b -> p (a b)"),
                        in_=ps1[:, i * T:(i + 1) * T],
                        func=AF.Exp, bias=nmax1[:, i:i + 1], scale=1.0,
                    )
                # wei^T via PE (both heads batched into one PSUM tile + copy)
                ps_t = ps_st.tile([P, 1024], BF16, tag="St", name=f"tw_{s}_{pr}")
                pstv = ps_t[:, 0:2 * CCH * P].rearrange(
                    "p (i c t) -> p i c t", i=2, c=CCH)
                for i in range(2):
                    for cc in range(CCH):
                        nc.tensor.transpose(pstv[:, i, cc, :], wei[:, i, cc, :],
                                            iden_sb)
                wT = atp.tile([P, 2, CCH, P], BF16, tag="wT",
                              name=f"wT_{s}_{pr}")
                nc.vector.tensor_copy(out=wT, in_=pstv)
                # att (+ rowsum in col 64 of each head's group)
                v_ext = d["v"]
                pa0, pa1 = d["ps_att"]
                for i in range(2):
                    hh = 2 * pr + i
                    o0 = pa0[:, hh * VW:hh * VW + HD + 1]
                    nc.tensor.matmul(o0, wT[:, i, 0, :],
                                     v_ext[:, 0, hh, 0:HD + 1],
                                     start=True, stop=True)
                    o1 = pa1[:, hh * VW:hh * VW + HD + 1]
                    for cs in range(2):
                        nc.tensor.matmul(o1, wT[:, i, 1 + cs, :],
                                         v_ext[:, cs, hh, 0:HD + 1],
                                         start=(cs == 0), stop=(cs == 1))

            def c1_tile(s, j):
                """Normalize att by PE row-sums, att^T, proj, residual, LN2,
                h2^T for token tile j."""
                d = st[s]
                pa = d["ps_att"][j]
                rs = stp.tile([P, H], F32, tag=f"rs{j}")
                nc.vector.reciprocal(
                    out=rs,
                    in_=pa[:, 0:H * VW].rearrange("p (h c) -> p h c", h=H)[:, :, HD:HD + 1],
                )
                att_sb = atp.tile([P, C], BF16, tag="att", bufs=3,
                                  name=f"att_{s}_{j}")
                for pr in range(NPAIR):
                    sl = att_sb[:, pr * P:(pr + 1) * P].rearrange(
                        "p (i c) -> p i c", i=2
                    )
                    s0 = pa[:, 2 * pr * VW:(2 * pr + 2) * VW].rearrange(
                        "p (i c) -> p i c", i=2
                    )[:, :, 0:HD]
                    nc.vector.tensor_tensor(
                        out=sl, in0=s0,
                        in1=bcast(rs[:, 2 * pr:2 * pr + 2], HD), op=OP.mult
                    )
                attT = ap.tile([P, CCH, P], BF16, tag="attT", bufs=3,
                               name=f"attT_{s}_{j}")
                transpose3(att_sb, attT, CCH, f"att_{s}_{j}", on_act=True)

                ps_sa = ps_w.tile([P, 512], F32, tag="w", name=f"ps_sa{j}_{s}")
                for c in range(CCH):
                    nc.tensor.matmul(
                        ps_sa[:, 0:C], attT[:, c, :], wp_sb[:, c, :],
                        start=(c == 0), stop=(c == CCH - 1 and not has_bp),
                    )
                if has_bp:
                    nc.tensor.matmul(ps_sa[:, 0:C], ones_sb, bext_sb[:, 1, :],
                                     start=False, stop=True)
                x2_i = xp.tile([P, C], F32, tag="x2", name=f"x2_{s}_{j}")
                nc.vector.tensor_tensor(out=x2_i, in0=ps_sa[:, 0:C],
                                        in1=d["x"][j], op=OP.add)
                d.setdefault("x2", {})[j] = x2_i

                if j == 0:
                    d["h2T"] = ap.tile([P, 4, T], FP8, tag="h2T", bufs=3,
                                       name=f"h2T_{s}")
                    if s < 3:
                        # chunk 3 pairs with w1's zero rows (DoubleRow K pad);
                        # slots rotate 2-deep, so two memsets cover all seqs
                        nc.vector.memset(d["h2T"][:, 3, :], 0.0)
                h2_i = ap.tile([P, C], BF16, tag="h2", name=f"h2_{s}_{j}")
                layer_norm(x2_i, h2_i)
                transpose3(h2_i, d["h2T"][:, 0:3, j * P:(j + 1) * P], CCH,
                           f"h2_{s}_{j}", on_act=True)

            def c2_ffn1(s, g2):
                """FFN1 group pair g2 (two 128-col groups), bias+ReLU."""
                d = st[s]
                if g2 == 0:
                    d["aT"] = ffp.tile([P, NFF, T], FP8, tag="aT",
                                       name=f"aT_{s}")
                ps_a = ps_w.tile([P, 512], F32, tag="w", name=f"psa_{s}_{g2}")
                for i in range(2):
                    g = 2 * g2 + i
                    for kt in range(2):
                        nc.tensor.matmul(
                            ps_a[:, i * T:(i + 1) * T],
                            w1_sb[:, 2 * kt:2 * kt + 2, g * P:(g + 1) * P],
                            d["h2T"][:, 2 * kt:2 * kt + 2, :],
                            start=(kt == 0), stop=(kt == 1),
                            perf_mode=mybir.MatmulPerfMode.DoubleRow,
                        )
                # bias+ReLU; alternate ACT/DVE to balance engine load
                if has_b1:
                    for i in range(2):
                        g = 2 * g2 + i
                        if g2 % 2 == 0:
                            nc.scalar.activation(
                                out=d["aT"][:, g, :],
                                in_=ps_a[:, i * T:(i + 1) * T],
                                func=AF.Relu, bias=b1_sb[:, g:g + 1], scale=1.0,
                            )
                        else:
                            nc.vector.tensor_scalar(
                                out=d["aT"][:, g, :],
                                in0=ps_a[:, i * T:(i + 1) * T],
                                scalar1=b1_sb[:, g:g + 1], scalar2=0.0,
                                op0=OP.add, op1=OP.max,
                            )
                elif g2 % 2 == 0:
                    nc.scalar.activation(
                        out=d["aT"][:, 2 * g2:2 * g2 + 2, :],
                        in_=ps_a[:, 0:2 * T].rearrange("p (i t) -> p i t", i=2),
                        func=AF.Relu, bias=0.0, scale=1.0,
                    )
                else:
                    nc.vector.tensor_scalar(
                        out=d["aT"][:, 2 * g2:2 * g2 + 2, :],
                        in0=ps_a[:, 0:2 * T].rearrange("p (i t) -> p i t", i=2),
                        scalar1=0.0, scalar2=None, op0=OP.max,
                    )

            def c2_ffn2(s, j):
                d = st[s]
                ps_y = ps_w.tile([P, 512], F32, tag="w", name=f"ps_y{j}_{s}")
                for g3 in range(NFF // 2):
                    nc.tensor.matmul(
                        ps_y[:, 0:C],
                        d["aT"][:, 2 * g3:2 * g3 + 2, j * P:(j + 1) * P],
                        w2_sb[:, 2 * g3:2 * g3 + 2, :],
                        start=(g3 == 0), stop=(g3 == NFF // 2 - 1 and not has_b2),
                        perf_mode=mybir.MatmulPerfMode.DoubleRow,
                    )
                if has_b2:
                    nc.tensor.matmul(ps_y[:, 0:C], ones_sb, bext_sb[:, 2, :],
                                     start=False, stop=True)
                o_i = op_.tile([P, C], F32, tag="o", name=f"o_{s}_{j}")
                # undo the fp8 weight pre-scales (64*64 = 2^12)
                nc.vector.scalar_tensor_tensor(
                    out=o_i, in0=ps_y[:, 0:C], scalar=1.0 / (FF_SCALE * FF_SCALE),
                    in1=d["x2"][j], op0=OP.mult, op1=OP.add,
                )
                it = 2 * s + j
                nc.gpsimd.dma_start(out=out_h[it * P:(it + 1) * P, :], in_=o_i)

            # ---------------- pipelined emission ----------------
            # Three-deep skew: iteration s emits attention(s) interleaved
            # with proj/LN2 of s-1, FFN of s-2, and LN1/QKV of s+1. Dense
            # matmul groups from older sequences pad every latency window of
            # the softmax/LN chains, keeping the PE stream saturated (HAM at
            # 2.4 GHz needs near-continuous PE activity).
            x_prefetch(0)
            a1_tile(0, 0)
            a1_tile(0, 1)
            x_prefetch(1)
            a2_v(0)
            a2_qk(0, "q")
            a2_qk(0, "k")
            for s in range(B_SH + 2):
                att = s if s < B_SH else None
                prv = s - 1 if 0 <= s - 1 < B_SH else None
                pv2 = s - 2 if 0 <= s - 2 < B_SH else None
                nxt = s + 1 if s + 1 < B_SH else None
                if s + 2 < B_SH:
                    x_prefetch(s + 2)
                if nxt is not None:
                    a1_tile(nxt, 0)
                    a1_tile(nxt, 1)
                if att is not None:
                    b_pair(att, 0)
                if pv2 is not None:
                    c2_ffn1(pv2, 0)
                    c2_ffn1(pv2, 1)
                if prv is not None:
                    c1_tile(prv, 0)
                if nxt is not None:
                    a2_v(nxt)
                if att is not None:
                    b_pair(att, 1)
                if pv2 is not None:
                    c2_ffn1(pv2, 2)
                    c2_ffn1(pv2, 3)
                if prv is not None:
                    c1_tile(prv, 1)
                if nxt is not None:
                    a2_qk(nxt, "q")
                if att is not None:
                    b_pair(att, 2)
                if pv2 is not None:
                    c2_ffn1(pv2, 4)
                    c2_ffn1(pv2, 5)
                if nxt is not None:
                    a2_qk(nxt, "k")
                if pv2 is not None:
                    c2_ffn2(pv2, 0)
                    c2_ffn2(pv2, 1)

    _hoist_extra_waits(nc)
    return nc


def _prep_weights(inputs):
    f32 = np.float32
    g1 = inputs["ln1_g"].astype(f32)
    b1l = inputs["ln1_b"].astype(f32)
    g2 = inputs["ln2_g"].astype(f32)
    b2l = inputs["ln2_b"].astype(f32)
    wq, wk, wv = (inputs[k].astype(f32) for k in ("wq", "wk", "wv"))
    w1 = inputs["w1"].astype(f32)

    # fold LN gains/biases + attention scale
    wq_f = wq * g1[None, :, None] * SCALE          # [H, C, hd]
    bq = SCALE * np.einsum("c,hcd->hd", b1l, wq)   # [H, hd]
    wk_f = wk * g1[None, :, None]
    bk = np.einsum("c,hcd->hd", b1l, wk)
    wv_f = wv * g1[None, :, None]
    bv = np.einsum("c,hcd->hd", b1l, wv)
    w1_f = np.zeros((4 * P, DFF), f32)
    w1_f[:C] = w1 * g2[:, None] * FF_SCALE
    b1f = (inputs["b1"].astype(f32) + b2l @ w1) * FF_SCALE

    # head-major column layout [C, H*hd]
    to_mat = lambda w: np.ascontiguousarray(w.transpose(1, 0, 2).reshape(C, C))
    d = {
        "wq_m": to_mat(wq_f).astype(_BF),
        "wk_m": to_mat(wk_f).astype(_BF),
        "wv_m": to_mat(wv_f).astype(_BF),
        "wp_m": np.ascontiguousarray(inputs["w_proj"].astype(f32)).astype(_BF),
        "w1_m": np.ascontiguousarray(w1_f).astype(_F8),
        "w2_m": np.ascontiguousarray(inputs["w2"].astype(f32) * FF_SCALE).astype(_F8),
        "bq_v": np.ascontiguousarray(bq.reshape(C)).astype(f32),
        "bk_v": np.ascontiguousarray(bk.reshape(C)).astype(f32),
        "b1_v": np.ascontiguousarray(b1f).astype(f32),
        "mask_m": np.triu(np.full((P, P), -1e9, dtype=f32), k=1),
        "tri_m": np.tril(np.ones((P, P), dtype=f32)).astype(_BF),
        "iden_m": np.eye(P, dtype=f32).astype(_BF),
    }
    bv_r = bv.reshape(C)
    bp_r = inputs["b_proj"].astype(f32)
    b2_r = inputs["b2"].astype(f32) * FF_SCALE * FF_SCALE
    d["bext_v"] = np.stack([bv_r, bp_r, b2_r]).astype(_BF)
    flags = (bool(np.any(bv_r)), bool(np.any(bp_r)), bool(np.any(b2_r)),
             bool(np.any(b1f)),
             bool(np.any(d["bq_v"])) or bool(np.any(d["bk_v"])))
    return d, flags


def kernel(**inputs) -> np.ndarray:
    x = np.ascontiguousarray(inputs["x"].astype(np.float32))
    weights, flags = _prep_weights(inputs)

    if flags not in _CACHE:
        _CACHE[flags] = _build(*flags)
    nc = _CACHE[flags]

    xs = x.reshape(N_CORES, TOK, C)
    in_maps = [dict(weights, x=np.ascontiguousarray(xs[i])) for i in range(N_CORES)]
    import os

    kwargs = {}
    if os.environ.get("BASS_PROF"):
        kwargs = {"trace": True, "trace_cores": [0]}
    res = run_bass_kernel_spmd(nc, in_maps, list(range(N_CORES)), **kwargs)
    globals()["LAST_RESULTS"] = res
    out = np.stack([res.results[i]["out"] for i in range(N_CORES)])
    return out.reshape(B, T, C).astype(np.float32)
